# revision 32
# baseline (speedup 1.0000x reference)
"""PrRoIPool (Precise RoI Pooling) Trainium2 Bass kernel.

Problem: features [2, 256, 100, 100] f32, rois [256, 5] f32 ->
out [256, 256, 7, 7] f32 where
  out[n,c,p,q] = (1/area) * sum_{h,w} F[bi,c,h,w] * wy[n,p,h] * wx[n,q,w]
with wy/wx the exact integrals of the bilinear-interp hat functions over
each pooling bin (separable).

Strategy (8 NeuronCores, SPMD):
  - Host: compute hat-integral weights wy [N,7,H], wx [N,7,W] (tiny:
    ~0.004%% of total FLOPs), fold 1/bin_h into wy and 1/bin_w into wx.
  - Shard ROIs by batch image: cores 0-3 take batch-0 ROIs, cores 4-7
    batch-1 (S slots per core, zero-padded). Each core holds the full
    feature image of its batch, pre-transposed to [W, C, H] fp16.
  - Stage A (per channel c): T1[h, (s,q)] = F[c].T @ wx  -- PE matmul,
    stationary [w=100, h=100], moving [w=100, S*7], fp32 PSUM, then
    cast-copy PSUM->SBUF fp16 (Vector/Scalar engines, rate-balanced).
  - Stage B (per ROI slot s, per 64-channel chunk j):
    out[p, (c',q)] = wy_s.T @ T1[:, s, 64j:64j+64, :]  -- 4 ROIs packed
    into one PSUM bank at partition offsets 0/32/64/96 via matmul
    tile_position; the 4 col-tiled matmuls execute concurrently.
  - DMA staged outputs to DRAM; host reassembles [N, C, 7, 7].

The pipeline is paced by PSUM->SBUF evacuation: fp32-source copies run
at 1 elem/cycle on DVE (0.96 GHz) / ACT (1.2 GHz) only (DMA and GPSIMD
cannot read PSUM), so stage A copies (~61K free-elems) + stage B copies
are split across both engines by measured per-op cost.
"""

import sys

if "/opt/trn_rl_repo" not in sys.path:
    sys.path.insert(0, "/opt/trn_rl_repo")

import numpy as np

POOLED = 7
SPATIAL_SCALE = 0.0625
B, C, H, W = 2, 256, 100, 100
N_CORES = 8
CORES_PER_BATCH = 4
S_CAP = 36  # max ROI slots per core (stage-A PSUM slot is half a bank)
CHUNK = 16  # channels per feature DMA

_prog_cache = {}


def _hat_cdf(u):
    return np.where(
        u <= 0.0,
        0.5 * np.clip(u + 1.0, 0.0, 1.0) ** 2,
        1.0 - 0.5 * np.clip(1.0 - u, 0.0, 1.0) ** 2,
    )


def _bin_weights(lo, hi, size):
    # [N, P] bounds -> [N, P, size] integral of hat centered at each index
    idx = np.arange(size, dtype=lo.dtype)
    return _hat_cdf(hi[..., None] - idx) - _hat_cdf(lo[..., None] - idx)


def _host_weights(rois):
    """Per-ROI separable weights with 1/area folded in. float32."""
    r = rois.astype(np.float64)
    x1 = r[:, 1] * SPATIAL_SCALE
    y1 = r[:, 2] * SPATIAL_SCALE
    x2 = r[:, 3] * SPATIAL_SCALE
    y2 = r[:, 4] * SPATIAL_SCALE
    bw = (x2 - x1) / POOLED
    bh = (y2 - y1) / POOLED
    ph = np.arange(POOLED, dtype=np.float64)
    ylo = y1[:, None] + ph * bh[:, None]
    yhi = ylo + bh[:, None]
    xlo = x1[:, None] + ph * bw[:, None]
    xhi = xlo + bw[:, None]
    wy = _bin_weights(ylo, yhi, H)  # [N, 7, H]
    wx = _bin_weights(xlo, xhi, W)  # [N, 7, W]
    # reference: out = einsum / max(area,1e-12) where area = bw*bh, zeroed
    # if area <= 0. Fold 1/bh into wy, 1/bw into wx (area > 0 case).
    ok = (bw * bh) > 0.0
    inv_bh = np.where(ok, 1.0 / np.maximum(bh, 1e-12), 0.0)
    inv_bw = np.where(ok, 1.0 / np.maximum(bw, 1e-12), 0.0)
    wy = wy * inv_bh[:, None, None]
    wx = wx * inv_bw[:, None, None]
    return wy.astype(np.float32), wx.astype(np.float32)


def _build_program(S):
    """Bass/Tile SPMD program for S ROI slots per core. Cached per S."""
    from contextlib import ExitStack

    from concourse import bacc, mybir
    import concourse.tile as tile

    f16 = mybir.dt.float16
    f32 = mybir.dt.float32
    SQ = S * POOLED
    assert SQ <= 256  # stage-A psum: 2 channel slots of 256 f32 = 1 bank

    nc = bacc.Bacc("TRN2", target_bir_lowering=False, debug=False,
                   num_devices=N_CORES)
    fwt = nc.dram_tensor("fwt", [W, C, H], f16, kind="ExternalInput")
    wxt = nc.dram_tensor("wxt", [W, SQ], f16, kind="ExternalInput")
    # wyt is padded to 32 cols per slot (25 zero) so each stage-B matmul
    # writes a full 32-partition PSUM block (cost is N-driven, M is free).
    wyt = nc.dram_tensor("wyt", [H, S * 32], f16, kind="ExternalInput")
    # output staged fp16, 4 ROIs per 128-partition block (rows 0-6/32-38/
    # 64-70/96-102 valid, rest zero), one DMA per (group, channel-half)
    NG = -(-S // 4)
    out = nc.dram_tensor("out", [NG, 2, 128, 2, 448], f16,
                         kind="ExternalOutput")

    # feature-chunk schedule: 8-ch chunks first (per-DMA latency is ~2-4us;
    # too-small early chunks starve the cold-clock PE), 16-ch after; chunk
    # boundaries align with 64-ch quarters
    chunks = []
    for q_sizes in ([8, 8, 16, 16, 16], [16, 16, 16, 16],
                    [16, 16, 16, 16], [16, 16, 16, 16]):
        chunks.extend(q_sizes)
    starts = np.cumsum([0] + chunks).tolist()

    # copy-engine load balancer: measured per-op ns cost (fp32 PSUM src,
    # ~1 elem/cycle: DVE 0.96 GHz + 120cyc overhead, ACT ~1.2 + 172cyc)
    V_A, S_A = 1148.0, 1053.0   # stage-A copy [100, 4*SQ]
    V_B, S_B = 610.0, 629.0     # stage-B copy [<=128, 448]
    V_H, S_H = 358.0, 333.0     # half stage-B copy [<=128, 224]

    with tile.TileContext(nc) as tc, ExitStack() as ctx:
        sb = ctx.enter_context(tc.tile_pool(name="sb", bufs=1))
        fw_pool = ctx.enter_context(tc.tile_pool(name="fw", bufs=4))
        pa_pool = ctx.enter_context(tc.tile_pool(name="pa", bufs=3,
                                                 space="PSUM"))
        pb_pool = ctx.enter_context(tc.tile_pool(name="pb", bufs=2,
                                                 space="PSUM"))
        stg_pool = ctx.enter_context(tc.tile_pool(name="stg", bufs=NG + 1))

        wx_t = sb.tile([W, SQ], f16, tag="wx")
        nc.sync.dma_start(out=wx_t[:], in_=wxt[:])
        wy_t = sb.tile([H, S * 32], f16, tag="wy")

        # warmup scratch: dense matmuls around the first stage-A group flip
        # the HAM clock gate to 8/8 (PE runs at 1.2 GHz until it sees a
        # ~3.4us window of sustained activity; everything after runs 2x)
        warm = sb.tile([128, 640], f16, tag="warm")
        nc.vector.memset(warm[:], 0.0)

        def warmup(n):
            wps = pb_pool.tile([128, 512], f32, tag="pb")
            for _ in range(n):
                nc.tensor.matmul(wps[:, 0:512], lhsT=warm[:, 0:128],
                                 rhs=warm[:, 128:640])

        # per-quarter T1 tiles so stage-B reads of quarter j never create
        # false WAR deps against stage-A writes of quarter j+1
        t1 = [sb.tile([H, S, 64, POOLED], f16, tag=f"t1_{j}",
                      name=f"t1_{j}") for j in range(4)]

        eng_load = [0.0, 0.0]  # accumulated ns on [vector, scalar]
        a_flip = [0]

        def copy(dst, src, cost_v, cost_s, strict=None):
            # strict alternation for the pa-rotation-critical stage-A
            # copies (deterministic release latency); greedy load balance
            # for the rest
            if strict is not None:
                use_v = strict
            else:
                use_v = eng_load[0] + cost_v <= eng_load[1] + cost_s
            if use_v:
                nc.vector.tensor_copy(dst, src)
                eng_load[0] += cost_v
            else:
                nc.scalar.copy(dst, src)
                eng_load[1] += cost_s

        def copy_a(dst, src):
            copy(dst, src, V_A, S_A, strict=(a_flip[0] % 2 == 0))
            a_flip[0] += 1

        def keepalive(cols=64):
            # cheap PE touch (32-col LDW + small matmul) to hold the HAM
            # clock gate at 8/8 through copy-paced gaps
            ka = pb_pool.tile([128, 512], f32, tag="pb")
            nc.tensor.matmul(ka[0:32, 0:cols], lhsT=warm[:, 0:32],
                             rhs=warm[:, 128:128 + cols])

        stgs = {}
        tail_pb = [None]
        # output DMAs alternate between the gpsimd and sync queues so the
        # ~650ns per-issue serialization never piles up on one queue
        dma_n = [0]

        def out_dma(dst_ap, src_ap):
            eng = nc.gpsimd if dma_n[0] % 2 == 0 else nc.sync
            dma_n[0] += 1
            eng.dma_start(out=dst_ap, in_=src_ap)

        def emit_b_group(j, g, half=None):
            # stage-B for quarter j, 4-ROI group g: 4 col-tiled matmuls at
            # PSUM partition offsets 0/32/64/96 (concurrent execution).
            # half None: all 64 channels; 0/1: 32-channel half (224 cols),
            # used to drain most of the last quarter inside the pipeline.
            hf, jj = divmod(j, 2)
            if jj == 0:
                stgs[g] = stg_pool.tile([128, 2, 448], f16, tag="stg",
                                        name=f"stg_{hf}_{g}")
            stg = stgs[g]
            g0 = 4 * g
            rois_g = list(range(g0, min(g0 + 4, S)))
            hi = 32 * len(rois_g)
            c0, c1 = (32 * half, 32 * half + 32) if half in (0, 1) \
                else (0, 64)
            x0, x1 = c0 * POOLED, c1 * POOLED
            if half == "tail" and g % 2 == 0:
                pa_t = pa_pool.tile([128, 4, 256], f32, tag="pa")
                pb = pa_t.rearrange("h a b -> h (a b)")
            else:
                pb = pb_pool.tile([128, 512], f32, tag="pb")
            for i, s in enumerate(rois_g):
                nc.tensor.matmul(
                    pb[32 * i:32 * i + 32, 0:x1 - x0],
                    lhsT=wy_t[:, s * 32:(s + 1) * 32],
                    rhs=t1[j][:, s, c0:c1, :],
                    tile_position=(0, 32 * i),
                )
            cv, cs = (V_H, S_H) if half in (0, 1) else (V_B, S_B)
            if half == "tail" and g == NG - 1:
                # last group: split copy over both engines and DMA over
                # both queues so the final drain starts earlier
                copy(stg[0:hi, jj, x0:224], pb[0:hi, 0:224], V_H, S_H,
                     strict=True)
                copy(stg[0:hi, jj, 224:448], pb[0:hi, 224:448], V_H, S_H,
                     strict=False)
                out_dma(out[g, hf, 0:hi, jj, 0:224],
                        stg[0:hi, jj, 0:224])
                out_dma(out[g, hf, 0:hi, jj, 224:448],
                        stg[0:hi, jj, 224:448])
                return
            if half == "tail":
                copy(stg[0:hi, jj, x0:x1], pb[0:hi, 0:x1 - x0], cv, cs,
                     strict=(g % 2 == 0))
            else:
                # parity chosen so B-copies land more often on the
                # cheaper Scalar engine, evening total busy across V/S
                copy(stg[0:hi, jj, x0:x1], pb[0:hi, 0:x1 - x0], cv, cs,
                     strict=(a_flip[0] % 2 == 0))
            # DMA each finished piece immediately so output traffic spreads
            # across the whole pipeline instead of piling up in the tail
            out_dma(out[g, hf, 0:hi, jj, x0:x1], stg[0:hi, jj, x0:x1])

        # chunk DMAs: first four pre-issued across idle engine queues (the
        # ~790ns per-issue serialization on one queue starves stage A);
        # later chunks prefetched two ahead on sync
        fw_tiles = {}

        def issue_chunk(k, eng):
            if k >= len(chunks) or k in fw_tiles:
                return
            t = fw_pool.tile([W, CHUNK, H], f16, tag="fw")
            eng.dma_start(out=t[:, 0:chunks[k], :],
                          in_=fwt[:, starts[k]:starts[k] + chunks[k], :])
            fw_tiles[k] = t

        issue_chunk(0, nc.scalar)
        issue_chunk(1, nc.gpsimd)
        issue_chunk(2, nc.gpsimd)
        issue_chunk(3, nc.gpsimd)
        # wy is large (S*32*H fp16) and first needed by stage B a quarter
        # in; issuing it after the early chunks keeps chunk0's completion
        # (which gates the whole copy chain) off its shadow
        nc.gpsimd.dma_start(out=wy_t[:], in_=wyt[:])

        # warmup runs first (forced via high_priority) while the early
        # chunks land; the first stage-A group follows as soon as chunk0
        # is in SBUF so the copy chain (the pipeline pacer) starts early
        with tc.high_priority():
            warmup(9)
        cur = fw_tiles[0]
        coff = 0
        pa = pa_pool.tile([128, 4, 256], f32, tag="pa")
        for c in range(4):
            nc.tensor.matmul(pa[0:H, c, 0:SQ], lhsT=cur[:, c, :],
                             rhs=wx_t[:])
        copy_a(t1[0][:, :, 0:4, :],
               pa[0:H, :, 0:SQ].rearrange("h c (s q) -> h s c q",
                                          q=POOLED))

        chunk_idx = 1
        for j in range(4):
            # ---- Stage A quarter: T1_j[h, s, c', q] = F[c].T @ wx ----
            # with the previous quarter's stage-B groups interleaved every
            # few channels to fill the copy-paced gaps on the PE
            pending_b = list(range(NG)) if j > 0 else []
            for ci, c in enumerate(range(64 * j, 64 * (j + 1))):
                if ci == 0 and j == 0:
                    continue  # channels 0-3 done above
                if c == starts[chunk_idx]:
                    cur = fw_tiles[chunk_idx]
                    coff = c
                    issue_chunk(chunk_idx + 2, nc.sync)
                    chunk_idx += 1
                if c % 4 == 0:
                    pa = pa_pool.tile([128, 4, 256], f32, tag="pa")
                if j == 0 and ci < 4:
                    continue
                nc.tensor.matmul(
                    pa[0:H, c % 4, 0:SQ],
                    lhsT=cur[:, c - coff, :],
                    rhs=wx_t[:],
                )
                if c % 4 == 3:
                    src = pa[0:H, :, 0:SQ].rearrange(
                        "h c (s q) -> h s c q", q=POOLED)
                    dst = t1[j][:, :, (c - 3) % 64:(c - 3) % 64 + 4, :]
                    copy_a(dst, src)
                if ci >= 13 and (ci - 13) % 6 == 0 and pending_b:
                    emit_b_group(j - 1, pending_b.pop(0))
                elif j == 0 and ci % 8 == 4:
                    # no stage-B work yet: keepalive matmul to hold the
                    # HAM clock gate at 8/8 through the copy-paced gaps
                    keepalive(512)
            for g in pending_b:
                emit_b_group(j - 1, g)
            if j < 3:
                # bridge keepalive over the quarter boundary (the next
                # quarter's first B group waits on this quarter's last
                # copy; a PE hole here trips the HAM re-throttle)
                keepalive(256)
        # last quarter's stage B (tail)
        keepalive(256)
        for g in range(NG):
            emit_b_group(3, g, half="tail")
            if g % 3 == 2:
                keepalive()

    nc.compile()
    return nc


def _plan_shards(bi, n_rois):
    """Assign ROI indices to (wave, core, slot). Returns S and a list of
    per-wave assignment arrays of shape [N_CORES, S] (-1 = padding)."""
    groups = [np.where(bi == b)[0] for b in range(B)]
    need = max((len(g) + CORES_PER_BATCH - 1) // CORES_PER_BATCH
               for g in groups)
    need = max(need, 1)
    S = min(need, S_CAP)
    per_wave_cap = S * CORES_PER_BATCH
    n_waves = max(-(-len(g) // per_wave_cap) for g in groups)
    waves = []
    for wv in range(n_waves):
        asg = np.full((N_CORES, S), -1, dtype=np.int64)
        for b in range(B):
            g = groups[b][wv * per_wave_cap:(wv + 1) * per_wave_cap]
            for k in range(CORES_PER_BATCH):
                chunk = g[k * S:(k + 1) * S]
                asg[b * CORES_PER_BATCH + k, :len(chunk)] = chunk
        waves.append(asg)
    return S, waves


def kernel(features, rois, _trace=False):
    from concourse.bass_utils import run_bass_kernel_spmd

    features = np.asarray(features, dtype=np.float32)
    rois = np.asarray(rois, dtype=np.float32)
    n_rois = rois.shape[0]
    bi = np.rint(rois[:, 0]).astype(np.int64)
    bi = np.where((bi >= 0) & (bi < B), bi, -1)

    wy, wx = _host_weights(rois)  # [N, 7, H] / [N, 7, W], 1/area folded
    S, waves = _plan_shards(bi, n_rois)

    if S not in _prog_cache:
        _prog_cache[S] = _build_program(S)
    nc = _prog_cache[S]

    # Features per batch, transposed to [W, C, H], fp16. Shared across the
    # 4 cores of each batch group.
    fwt = [np.ascontiguousarray(features[b].transpose(2, 0, 1))
           .astype(np.float16) for b in range(B)]

    out_full = np.zeros((n_rois, C, POOLED, POOLED), dtype=np.float32)
    exec_ns = None
    for asg in waves:
        in_maps = []
        for k in range(N_CORES):
            wxt = np.zeros((W, S * POOLED), dtype=np.float16)
            wyt = np.zeros((H, S * 32), dtype=np.float16)
            for s in range(S):
                r = asg[k, s]
                if r < 0:
                    continue
                # w[n, q, w-axis] -> [w-axis, s*7+q]
                wxt[:, s * POOLED:(s + 1) * POOLED] = \
                    wx[r].T.astype(np.float16)
                wyt[:, s * 32:s * 32 + POOLED] = \
                    wy[r].T.astype(np.float16)
            in_maps.append({
                "fwt": fwt[k // CORES_PER_BATCH],
                "wxt": wxt,
                "wyt": wyt,
            })
        res = run_bass_kernel_spmd(nc, in_maps, list(range(N_CORES)),
                                   trace=_trace)
        if res.exec_time_ns is not None:
            exec_ns = max(exec_ns or 0, res.exec_time_ns)
        for k in range(N_CORES):
            arr = res.results[k]["out"]  # [NG, 2, 128, 2, 448] f16
            for s in range(S):
                r = asg[k, s]
                if r < 0:
                    continue
                g, i = divmod(s, 4)
                # [hf, p, jj, c', q] -> [hf, jj, c', p, q] -> [C, 7, 7]
                blk = (arr[g, :, 32 * i:32 * i + POOLED]
                       .reshape(2, POOLED, 2, 64, POOLED)
                       .transpose(0, 2, 3, 1, 4)
                       .reshape(C, POOLED, POOLED))
                out_full[r] = blk.astype(np.float32)

    if _trace:
        kernel.last_exec_time_ns = exec_ns
    return out_full


# revision 33
# speedup vs baseline: 1.0699x; 1.0699x over previous
"""PrRoIPool (Precise RoI Pooling) Trainium2 Bass kernel.

Problem: features [2, 256, 100, 100] f32, rois [256, 5] f32 ->
out [256, 256, 7, 7] f32 where
  out[n,c,p,q] = (1/area) * sum_{h,w} F[bi,c,h,w] * wy[n,p,h] * wx[n,q,w]
with wy/wx the exact integrals of the bilinear-interp hat functions over
each pooling bin (separable).

Strategy (8 NeuronCores, SPMD):
  - Host: compute hat-integral weights wy [N,7,H], wx [N,7,W] (tiny:
    ~0.004%% of total FLOPs), fold 1/bin_h into wy and 1/bin_w into wx.
  - Shard ROIs by batch image: cores 0-3 take batch-0 ROIs, cores 4-7
    batch-1 (S slots per core, zero-padded). Each core holds the full
    feature image of its batch, pre-transposed to [W, C, H] fp16.
  - Stage A (per channel c): T1[h, (s,q)] = F[c].T @ wx  -- PE matmul,
    stationary [w=100, h=100], moving [w=100, S*7], fp32 PSUM, then
    cast-copy PSUM->SBUF fp16 (Vector/Scalar engines, rate-balanced).
  - Stage B (per ROI slot s, per 64-channel chunk j):
    out[p, (c',q)] = wy_s.T @ T1[:, s, 64j:64j+64, :]  -- 4 ROIs packed
    into one PSUM bank at partition offsets 0/32/64/96 via matmul
    tile_position; the 4 col-tiled matmuls execute concurrently.
  - DMA staged outputs to DRAM; host reassembles [N, C, 7, 7].

The pipeline is paced by PSUM->SBUF evacuation: fp32-source copies run
at 1 elem/cycle on DVE (0.96 GHz) / ACT (1.2 GHz) only (DMA and GPSIMD
cannot read PSUM), so stage A copies (~61K free-elems) + stage B copies
are split across both engines by measured per-op cost.
"""

import sys

if "/opt/trn_rl_repo" not in sys.path:
    sys.path.insert(0, "/opt/trn_rl_repo")

import numpy as np

POOLED = 7
SPATIAL_SCALE = 0.0625
B, C, H, W = 2, 256, 100, 100
N_CORES = 8
CORES_PER_BATCH = 4
S_CAP = 36  # max ROI slots per core (stage-A PSUM slot is half a bank)
CHUNK = 16  # channels per feature DMA

_prog_cache = {}


def _hat_cdf(u):
    return np.where(
        u <= 0.0,
        0.5 * np.clip(u + 1.0, 0.0, 1.0) ** 2,
        1.0 - 0.5 * np.clip(1.0 - u, 0.0, 1.0) ** 2,
    )


def _bin_weights(lo, hi, size):
    # [N, P] bounds -> [N, P, size] integral of hat centered at each index
    idx = np.arange(size, dtype=lo.dtype)
    return _hat_cdf(hi[..., None] - idx) - _hat_cdf(lo[..., None] - idx)


def _host_weights(rois):
    """Per-ROI separable weights with 1/area folded in. float32."""
    r = rois.astype(np.float64)
    x1 = r[:, 1] * SPATIAL_SCALE
    y1 = r[:, 2] * SPATIAL_SCALE
    x2 = r[:, 3] * SPATIAL_SCALE
    y2 = r[:, 4] * SPATIAL_SCALE
    bw = (x2 - x1) / POOLED
    bh = (y2 - y1) / POOLED
    ph = np.arange(POOLED, dtype=np.float64)
    ylo = y1[:, None] + ph * bh[:, None]
    yhi = ylo + bh[:, None]
    xlo = x1[:, None] + ph * bw[:, None]
    xhi = xlo + bw[:, None]
    wy = _bin_weights(ylo, yhi, H)  # [N, 7, H]
    wx = _bin_weights(xlo, xhi, W)  # [N, 7, W]
    # reference: out = einsum / max(area,1e-12) where area = bw*bh, zeroed
    # if area <= 0. Fold 1/bh into wy, 1/bw into wx (area > 0 case).
    ok = (bw * bh) > 0.0
    inv_bh = np.where(ok, 1.0 / np.maximum(bh, 1e-12), 0.0)
    inv_bw = np.where(ok, 1.0 / np.maximum(bw, 1e-12), 0.0)
    wy = wy * inv_bh[:, None, None]
    wx = wx * inv_bw[:, None, None]
    return wy.astype(np.float32), wx.astype(np.float32)


def _build_program(S):
    """Bass/Tile SPMD program for S ROI slots per core. Cached per S."""
    from contextlib import ExitStack

    from concourse import bacc, mybir
    import concourse.tile as tile

    f16 = mybir.dt.float16
    f32 = mybir.dt.float32
    SQ = S * POOLED
    assert SQ <= 256  # stage-A psum: 2 channel slots of 256 f32 = 1 bank

    nc = bacc.Bacc("TRN2", target_bir_lowering=False, debug=False,
                   num_devices=N_CORES)
    fwt = nc.dram_tensor("fwt", [W, C, H], f16, kind="ExternalInput")
    wxt = nc.dram_tensor("wxt", [W, SQ], f16, kind="ExternalInput")
    # wyt is padded to 32 cols per slot (25 zero) so each stage-B matmul
    # writes a full 32-partition PSUM block (cost is N-driven, M is free).
    wyt = nc.dram_tensor("wyt", [H, S * 32], f16, kind="ExternalInput")
    # output staged fp16, 4 ROIs per 128-partition block (rows 0-6/32-38/
    # 64-70/96-102 valid, rest zero), one DMA per (group, channel-half)
    NG = -(-S // 4)
    out = nc.dram_tensor("out", [NG, 2, 128, 2, 448], f16,
                         kind="ExternalOutput")

    # feature-chunk schedule: 8-ch chunks first (per-DMA latency is ~2-4us;
    # too-small early chunks starve the cold-clock PE), 16-ch after; chunk
    # boundaries align with 64-ch quarters
    chunks = []
    for q_sizes in ([8, 8, 16, 16, 16], [16, 16, 16, 16],
                    [16, 16, 16, 16], [16, 16, 16, 16]):
        chunks.extend(q_sizes)
    starts = np.cumsum([0] + chunks).tolist()

    # copy-engine load balancer: measured per-op ns cost (fp32 PSUM src,
    # ~1 elem/cycle: DVE 0.96 GHz + 120cyc overhead, ACT ~1.2 + 172cyc)
    V_A, S_A = 1148.0, 1053.0   # stage-A copy [100, 4*SQ]
    V_B, S_B = 610.0, 629.0     # stage-B copy [<=128, 448]
    V_H, S_H = 358.0, 333.0     # half stage-B copy [<=128, 224]

    with tile.TileContext(nc) as tc, ExitStack() as ctx:
        sb = ctx.enter_context(tc.tile_pool(name="sb", bufs=1))
        fw_pool = ctx.enter_context(tc.tile_pool(name="fw", bufs=4))
        pa_pool = ctx.enter_context(tc.tile_pool(name="pa", bufs=3,
                                                 space="PSUM"))
        pb_pool = ctx.enter_context(tc.tile_pool(name="pb", bufs=2,
                                                 space="PSUM"))
        stg_pool = ctx.enter_context(tc.tile_pool(name="stg", bufs=NG + 1))

        wx_t = sb.tile([W, SQ], f16, tag="wx")
        nc.sync.dma_start(out=wx_t[:], in_=wxt[:])
        wy_t = sb.tile([H, S * 32], f16, tag="wy")

        # warmup scratch: dense matmuls around the first stage-A group flip
        # the HAM clock gate to 8/8 (PE runs at 1.2 GHz until it sees a
        # ~3.4us window of sustained activity; everything after runs 2x)
        warm = sb.tile([128, 640], f16, tag="warm")
        nc.vector.memset(warm[:], 0.0)

        def warmup(n):
            wps = pb_pool.tile([128, 512], f32, tag="pb")
            for _ in range(n):
                nc.tensor.matmul(wps[:, 0:512], lhsT=warm[:, 0:128],
                                 rhs=warm[:, 128:640])

        # per-quarter T1 tiles so stage-B reads of quarter j never create
        # false WAR deps against stage-A writes of quarter j+1
        t1 = [sb.tile([H, S, 64, POOLED], f16, tag=f"t1_{j}",
                      name=f"t1_{j}") for j in range(4)]

        eng_load = [0.0, 0.0]  # accumulated ns on [vector, scalar]
        a_flip = [0]

        def copy(dst, src, cost_v, cost_s, strict=None):
            # strict alternation for the pa-rotation-critical stage-A
            # copies (deterministic release latency); greedy load balance
            # for the rest
            if strict is not None:
                use_v = strict
            else:
                use_v = eng_load[0] + cost_v <= eng_load[1] + cost_s
            if use_v:
                nc.vector.tensor_copy(dst, src)
                eng_load[0] += cost_v
            else:
                nc.scalar.copy(dst, src)
                eng_load[1] += cost_s

        def copy_a(dst, src):
            copy(dst, src, V_A, S_A, strict=(a_flip[0] % 2 == 0))
            a_flip[0] += 1

        def keepalive(cols=64):
            # cheap PE touch (32-col LDW + small matmul) to hold the HAM
            # clock gate at 8/8 through copy-paced gaps
            ka = pb_pool.tile([128, 512], f32, tag="pb")
            nc.tensor.matmul(ka[0:32, 0:cols], lhsT=warm[:, 0:32],
                             rhs=warm[:, 128:128 + cols])

        stgs = {}
        tail_pb = [None]
        # output DMAs alternate between the gpsimd and sync queues so the
        # ~650ns per-issue serialization never piles up on one queue
        dma_n = [0]

        def out_dma(dst_ap, src_ap):
            eng = nc.gpsimd if dma_n[0] % 2 == 0 else nc.sync
            dma_n[0] += 1
            eng.dma_start(out=dst_ap, in_=src_ap)

        def emit_b_group(j, g, half=None):
            # stage-B for quarter j, 4-ROI group g: 4 col-tiled matmuls at
            # PSUM partition offsets 0/32/64/96 (concurrent execution).
            # half None: all 64 channels; 0/1: 32-channel half (224 cols),
            # used to drain most of the last quarter inside the pipeline.
            hf, jj = divmod(j, 2)
            if jj == 0:
                stgs[g] = stg_pool.tile([128, 2, 448], f16, tag="stg",
                                        name=f"stg_{hf}_{g}")
            stg = stgs[g]
            g0 = 4 * g
            rois_g = list(range(g0, min(g0 + 4, S)))
            hi = 32 * len(rois_g)
            c0, c1 = (32 * half, 32 * half + 32) if half in (0, 1) \
                else (0, 64)
            x0, x1 = c0 * POOLED, c1 * POOLED
            if half == "tail" and g % 2 == 0:
                pa_t = pa_pool.tile([128, 4, 256], f32, tag="pa")
                pb = pa_t.rearrange("h a b -> h (a b)")
            else:
                pb = pb_pool.tile([128, 512], f32, tag="pb")
            for i, s in enumerate(rois_g):
                nc.tensor.matmul(
                    pb[32 * i:32 * i + 32, 0:x1 - x0],
                    lhsT=wy_t[:, s * 32:(s + 1) * 32],
                    rhs=t1[j][:, s, c0:c1, :],
                    tile_position=(0, 32 * i),
                )
            cv, cs = (V_H, S_H) if half in (0, 1) else (V_B, S_B)
            if half == "tail" and g == NG - 1:
                # last group: split copy over both engines and DMA over
                # both queues so the final drain starts earlier
                copy(stg[0:hi, jj, x0:224], pb[0:hi, 0:224], V_H, S_H,
                     strict=True)
                copy(stg[0:hi, jj, 224:448], pb[0:hi, 224:448], V_H, S_H,
                     strict=False)
                out_dma(out[g, hf, 0:hi, jj, 0:224],
                        stg[0:hi, jj, 0:224])
                out_dma(out[g, hf, 0:hi, jj, 224:448],
                        stg[0:hi, jj, 224:448])
                return
            if half == "tail":
                copy(stg[0:hi, jj, x0:x1], pb[0:hi, 0:x1 - x0], cv, cs,
                     strict=(g % 2 == 0))
            else:
                # assign to the engine of the most recent stage-A copy:
                # its next pa-critical copy is furthest away, so this
                # B-copy never delays the pa rotation from the queue head
                copy(stg[0:hi, jj, x0:x1], pb[0:hi, 0:x1 - x0], cv, cs,
                     strict=(a_flip[0] % 2 == 1))
            # DMA each finished piece immediately so output traffic spreads
            # across the whole pipeline instead of piling up in the tail
            out_dma(out[g, hf, 0:hi, jj, x0:x1], stg[0:hi, jj, x0:x1])

        # chunk DMAs: first four pre-issued across idle engine queues (the
        # ~790ns per-issue serialization on one queue starves stage A);
        # later chunks prefetched two ahead on sync
        fw_tiles = {}

        def issue_chunk(k, eng):
            if k >= len(chunks) or k in fw_tiles:
                return
            t = fw_pool.tile([W, CHUNK, H], f16, tag="fw")
            eng.dma_start(out=t[:, 0:chunks[k], :],
                          in_=fwt[:, starts[k]:starts[k] + chunks[k], :])
            fw_tiles[k] = t

        issue_chunk(0, nc.scalar)
        issue_chunk(1, nc.gpsimd)
        issue_chunk(2, nc.gpsimd)
        issue_chunk(3, nc.gpsimd)
        # wy is large (S*32*H fp16) and first needed by stage B a quarter
        # in; issuing it after the early chunks keeps chunk0's completion
        # (which gates the whole copy chain) off its shadow
        nc.gpsimd.dma_start(out=wy_t[:], in_=wyt[:])

        # warmup runs first (forced via high_priority) while the early
        # chunks land; the first stage-A group follows as soon as chunk0
        # is in SBUF so the copy chain (the pipeline pacer) starts early
        with tc.high_priority():
            warmup(9)
        cur = fw_tiles[0]
        coff = 0
        pa = pa_pool.tile([128, 4, 256], f32, tag="pa")
        for c in range(4):
            nc.tensor.matmul(pa[0:H, c, 0:SQ], lhsT=cur[:, c, :],
                             rhs=wx_t[:])
        copy_a(t1[0][:, :, 0:4, :],
               pa[0:H, :, 0:SQ].rearrange("h c (s q) -> h s c q",
                                          q=POOLED))

        chunk_idx = 1
        for j in range(4):
            # ---- Stage A quarter: T1_j[h, s, c', q] = F[c].T @ wx ----
            # with the previous quarter's stage-B groups interleaved every
            # few channels to fill the copy-paced gaps on the PE
            pending_b = list(range(NG)) if j > 0 else []
            for ci, c in enumerate(range(64 * j, 64 * (j + 1))):
                if ci == 0 and j == 0:
                    continue  # channels 0-3 done above
                if c == starts[chunk_idx]:
                    cur = fw_tiles[chunk_idx]
                    coff = c
                    issue_chunk(chunk_idx + 2, nc.sync)
                    chunk_idx += 1
                if c % 4 == 0:
                    pa = pa_pool.tile([128, 4, 256], f32, tag="pa")
                if j == 0 and ci < 4:
                    continue
                nc.tensor.matmul(
                    pa[0:H, c % 4, 0:SQ],
                    lhsT=cur[:, c - coff, :],
                    rhs=wx_t[:],
                )
                if c % 4 == 3:
                    src = pa[0:H, :, 0:SQ].rearrange(
                        "h c (s q) -> h s c q", q=POOLED)
                    dst = t1[j][:, :, (c - 3) % 64:(c - 3) % 64 + 4, :]
                    copy_a(dst, src)
                if ci >= 13 and (ci - 13) % 6 == 0 and pending_b:
                    emit_b_group(j - 1, pending_b.pop(0))
                elif j == 0 and ci % 8 == 4:
                    # no stage-B work yet: keepalive matmul to hold the
                    # HAM clock gate at 8/8 through the copy-paced gaps
                    keepalive(512)
            for g in pending_b:
                emit_b_group(j - 1, g)
            if j < 3:
                # bridge keepalive over the quarter boundary (the next
                # quarter's first B group waits on this quarter's last
                # copy; a PE hole here trips the HAM re-throttle)
                keepalive(256)
        # last quarter's stage B (tail)
        keepalive(256)
        for g in range(NG):
            emit_b_group(3, g, half="tail")
            if g % 3 == 2:
                keepalive()

    nc.compile()
    return nc


def _plan_shards(bi, n_rois):
    """Assign ROI indices to (wave, core, slot). Returns S and a list of
    per-wave assignment arrays of shape [N_CORES, S] (-1 = padding)."""
    groups = [np.where(bi == b)[0] for b in range(B)]
    need = max((len(g) + CORES_PER_BATCH - 1) // CORES_PER_BATCH
               for g in groups)
    need = max(need, 1)
    S = min(need, S_CAP)
    per_wave_cap = S * CORES_PER_BATCH
    n_waves = max(-(-len(g) // per_wave_cap) for g in groups)
    waves = []
    for wv in range(n_waves):
        asg = np.full((N_CORES, S), -1, dtype=np.int64)
        for b in range(B):
            g = groups[b][wv * per_wave_cap:(wv + 1) * per_wave_cap]
            for k in range(CORES_PER_BATCH):
                chunk = g[k * S:(k + 1) * S]
                asg[b * CORES_PER_BATCH + k, :len(chunk)] = chunk
        waves.append(asg)
    return S, waves


def kernel(features, rois, _trace=False):
    from concourse.bass_utils import run_bass_kernel_spmd

    features = np.asarray(features, dtype=np.float32)
    rois = np.asarray(rois, dtype=np.float32)
    n_rois = rois.shape[0]
    bi = np.rint(rois[:, 0]).astype(np.int64)
    bi = np.where((bi >= 0) & (bi < B), bi, -1)

    wy, wx = _host_weights(rois)  # [N, 7, H] / [N, 7, W], 1/area folded
    S, waves = _plan_shards(bi, n_rois)

    if S not in _prog_cache:
        _prog_cache[S] = _build_program(S)
    nc = _prog_cache[S]

    # Features per batch, transposed to [W, C, H], fp16. Shared across the
    # 4 cores of each batch group.
    fwt = [np.ascontiguousarray(features[b].transpose(2, 0, 1))
           .astype(np.float16) for b in range(B)]

    out_full = np.zeros((n_rois, C, POOLED, POOLED), dtype=np.float32)
    exec_ns = None
    for asg in waves:
        in_maps = []
        for k in range(N_CORES):
            wxt = np.zeros((W, S * POOLED), dtype=np.float16)
            wyt = np.zeros((H, S * 32), dtype=np.float16)
            for s in range(S):
                r = asg[k, s]
                if r < 0:
                    continue
                # w[n, q, w-axis] -> [w-axis, s*7+q]
                wxt[:, s * POOLED:(s + 1) * POOLED] = \
                    wx[r].T.astype(np.float16)
                wyt[:, s * 32:s * 32 + POOLED] = \
                    wy[r].T.astype(np.float16)
            in_maps.append({
                "fwt": fwt[k // CORES_PER_BATCH],
                "wxt": wxt,
                "wyt": wyt,
            })
        res = run_bass_kernel_spmd(nc, in_maps, list(range(N_CORES)),
                                   trace=_trace)
        if res.exec_time_ns is not None:
            exec_ns = max(exec_ns or 0, res.exec_time_ns)
        for k in range(N_CORES):
            arr = res.results[k]["out"]  # [NG, 2, 128, 2, 448] f16
            for s in range(S):
                r = asg[k, s]
                if r < 0:
                    continue
                g, i = divmod(s, 4)
                # [hf, p, jj, c', q] -> [hf, jj, c', p, q] -> [C, 7, 7]
                blk = (arr[g, :, 32 * i:32 * i + POOLED]
                       .reshape(2, POOLED, 2, 64, POOLED)
                       .transpose(0, 2, 3, 1, 4)
                       .reshape(C, POOLED, POOLED))
                out_full[r] = blk.astype(np.float32)

    if _trace:
        kernel.last_exec_time_ns = exec_ns
    return out_full


# revision 37
# speedup vs baseline: 1.0727x; 1.0026x over previous
"""PrRoIPool (Precise RoI Pooling) Trainium2 Bass kernel.

Problem: features [2, 256, 100, 100] f32, rois [256, 5] f32 ->
out [256, 256, 7, 7] f32 where
  out[n,c,p,q] = (1/area) * sum_{h,w} F[bi,c,h,w] * wy[n,p,h] * wx[n,q,w]
with wy/wx the exact integrals of the bilinear-interp hat functions over
each pooling bin (separable).

Strategy (8 NeuronCores, SPMD):
  - Host: compute hat-integral weights wy [N,7,H], wx [N,7,W] (tiny:
    ~0.004%% of total FLOPs), fold 1/bin_h into wy and 1/bin_w into wx.
  - Shard ROIs by batch image: cores 0-3 take batch-0 ROIs, cores 4-7
    batch-1 (S slots per core, zero-padded). Each core holds the full
    feature image of its batch, pre-transposed to [W, C, H] fp16.
  - Stage A (per channel c): T1[h, (s,q)] = F[c].T @ wx  -- PE matmul,
    stationary [w=100, h=100], moving [w=100, S*7], fp32 PSUM, then
    cast-copy PSUM->SBUF fp16 (Vector/Scalar engines, rate-balanced).
  - Stage B (per ROI slot s, per 64-channel chunk j):
    out[p, (c',q)] = wy_s.T @ T1[:, s, 64j:64j+64, :]  -- 4 ROIs packed
    into one PSUM bank at partition offsets 0/32/64/96 via matmul
    tile_position; the 4 col-tiled matmuls execute concurrently.
  - DMA staged outputs to DRAM; host reassembles [N, C, 7, 7].

The pipeline is paced by PSUM->SBUF evacuation: fp32-source copies run
at 1 elem/cycle on DVE (0.96 GHz) / ACT (1.2 GHz) only (DMA and GPSIMD
cannot read PSUM), so stage A copies (~61K free-elems) + stage B copies
are split across both engines by measured per-op cost.
"""

import sys

if "/opt/trn_rl_repo" not in sys.path:
    sys.path.insert(0, "/opt/trn_rl_repo")

import numpy as np

POOLED = 7
SPATIAL_SCALE = 0.0625
B, C, H, W = 2, 256, 100, 100
N_CORES = 8
CORES_PER_BATCH = 4
S_CAP = 36  # max ROI slots per core (stage-A PSUM slot is half a bank)
CHUNK = 16  # channels per feature DMA

_prog_cache = {}


def _hat_cdf(u):
    return np.where(
        u <= 0.0,
        0.5 * np.clip(u + 1.0, 0.0, 1.0) ** 2,
        1.0 - 0.5 * np.clip(1.0 - u, 0.0, 1.0) ** 2,
    )


def _bin_weights(lo, hi, size):
    # [N, P] bounds -> [N, P, size] integral of hat centered at each index
    idx = np.arange(size, dtype=lo.dtype)
    return _hat_cdf(hi[..., None] - idx) - _hat_cdf(lo[..., None] - idx)


def _host_weights(rois):
    """Per-ROI separable weights with 1/area folded in. float32."""
    r = rois.astype(np.float64)
    x1 = r[:, 1] * SPATIAL_SCALE
    y1 = r[:, 2] * SPATIAL_SCALE
    x2 = r[:, 3] * SPATIAL_SCALE
    y2 = r[:, 4] * SPATIAL_SCALE
    bw = (x2 - x1) / POOLED
    bh = (y2 - y1) / POOLED
    ph = np.arange(POOLED, dtype=np.float64)
    ylo = y1[:, None] + ph * bh[:, None]
    yhi = ylo + bh[:, None]
    xlo = x1[:, None] + ph * bw[:, None]
    xhi = xlo + bw[:, None]
    wy = _bin_weights(ylo, yhi, H)  # [N, 7, H]
    wx = _bin_weights(xlo, xhi, W)  # [N, 7, W]
    # reference: out = einsum / max(area,1e-12) where area = bw*bh, zeroed
    # if area <= 0. Fold 1/bh into wy, 1/bw into wx (area > 0 case).
    ok = (bw * bh) > 0.0
    inv_bh = np.where(ok, 1.0 / np.maximum(bh, 1e-12), 0.0)
    inv_bw = np.where(ok, 1.0 / np.maximum(bw, 1e-12), 0.0)
    wy = wy * inv_bh[:, None, None]
    wx = wx * inv_bw[:, None, None]
    return wy.astype(np.float32), wx.astype(np.float32)


def _build_program(S):
    """Bass/Tile SPMD program for S ROI slots per core. Cached per S."""
    from contextlib import ExitStack

    from concourse import bacc, mybir
    import concourse.tile as tile

    f16 = mybir.dt.float16
    f32 = mybir.dt.float32
    SQ = S * POOLED
    assert SQ <= 256  # stage-A psum: 2 channel slots of 256 f32 = 1 bank

    nc = bacc.Bacc("TRN2", target_bir_lowering=False, debug=False,
                   num_devices=N_CORES)
    fwt = nc.dram_tensor("fwt", [W, C, H], f16, kind="ExternalInput")
    wxt = nc.dram_tensor("wxt", [W, SQ], f16, kind="ExternalInput")
    # wyt is padded to 32 cols per slot (25 zero) so each stage-B matmul
    # writes a full 32-partition PSUM block (cost is N-driven, M is free).
    wyt = nc.dram_tensor("wyt", [H, S * 32], f16, kind="ExternalInput")
    # output staged fp16, 4 ROIs per 128-partition block (rows 0-6/32-38/
    # 64-70/96-102 valid, rest zero), one DMA per (group, channel-half)
    NG = -(-S // 4)
    out = nc.dram_tensor("out", [NG, 2, 128, 2, 448], f16,
                         kind="ExternalOutput")

    # feature-chunk schedule: 8-ch chunks first (per-DMA latency is ~2-4us;
    # too-small early chunks starve the cold-clock PE), 16-ch after; chunk
    # boundaries align with 64-ch quarters
    chunks = []
    for q_sizes in ([8, 8, 16, 16, 16], [16, 16, 16, 16],
                    [16, 16, 16, 16], [16, 16, 16, 16]):
        chunks.extend(q_sizes)
    starts = np.cumsum([0] + chunks).tolist()

    # copy-engine load balancer: measured per-op ns cost (fp32 PSUM src,
    # ~1 elem/cycle: DVE 0.96 GHz + 120cyc overhead, ACT ~1.2 + 172cyc)
    V_A, S_A = 1148.0, 1053.0   # stage-A copy [100, 4*SQ]
    V_B, S_B = 610.0, 629.0     # stage-B copy [<=128, 448]
    V_H, S_H = 358.0, 333.0     # half stage-B copy [<=128, 224]

    with tile.TileContext(nc) as tc, ExitStack() as ctx:
        sb = ctx.enter_context(tc.tile_pool(name="sb", bufs=1))
        fw_pool = ctx.enter_context(tc.tile_pool(name="fw", bufs=4))
        pa_pool = ctx.enter_context(tc.tile_pool(name="pa", bufs=3,
                                                 space="PSUM"))
        pb_pool = ctx.enter_context(tc.tile_pool(name="pb", bufs=2,
                                                 space="PSUM"))
        stg_pool = ctx.enter_context(tc.tile_pool(name="stg", bufs=NG + 1))

        wx_t = sb.tile([W, SQ], f16, tag="wx")
        nc.sync.dma_start(out=wx_t[:], in_=wxt[:])
        wy_t = sb.tile([H, S * 32], f16, tag="wy")

        # warmup scratch: dense matmuls around the first stage-A group flip
        # the HAM clock gate to 8/8 (PE runs at 1.2 GHz until it sees a
        # ~3.4us window of sustained activity; everything after runs 2x)
        warm = sb.tile([128, 640], f16, tag="warm")
        nc.vector.memset(warm[:], 0.0)

        def warmup(n):
            wps = pb_pool.tile([128, 512], f32, tag="pb")
            for _ in range(n):
                nc.tensor.matmul(wps[:, 0:512], lhsT=warm[:, 0:128],
                                 rhs=warm[:, 128:640])

        # per-quarter T1 tiles so stage-B reads of quarter j never create
        # false WAR deps against stage-A writes of quarter j+1
        t1 = [sb.tile([H, S, 64, POOLED], f16, tag=f"t1_{j}",
                      name=f"t1_{j}") for j in range(4)]

        eng_load = [0.0, 0.0]  # accumulated ns on [vector, scalar]
        a_flip = [0]
        # stage-A copy schedule: strict V/S alternation (deterministic pa
        # release) but Scalar takes 3 extra turns (its copies are ~9%
        # cheaper, evening total busy). The S,S double-turns sit in the
        # cold-PE early phase where production, not copying, paces the
        # pa rotation. True = Vector.
        a_sched = []
        nxt = False
        for _k in range(80):
            if _k in (5, 11, 17):
                a_sched.append(False)
                continue
            a_sched.append(nxt)
            nxt = not nxt

        def copy(dst, src, cost_v, cost_s, strict=None):
            # strict alternation for the pa-rotation-critical stage-A
            # copies (deterministic release latency); greedy load balance
            # for the rest
            if strict is not None:
                use_v = strict
            else:
                use_v = eng_load[0] + cost_v <= eng_load[1] + cost_s
            if use_v:
                nc.vector.tensor_copy(dst, src)
                eng_load[0] += cost_v
            else:
                nc.scalar.copy(dst, src)
                eng_load[1] += cost_s

        def copy_a(dst, src):
            copy(dst, src, V_A, S_A, strict=a_sched[a_flip[0]])
            a_flip[0] += 1

        def keepalive(cols=64):
            # cheap PE touch (32-col LDW + small matmul) to hold the HAM
            # clock gate at 8/8 through copy-paced gaps
            ka = pb_pool.tile([128, 512], f32, tag="pb")
            nc.tensor.matmul(ka[0:32, 0:cols], lhsT=warm[:, 0:32],
                             rhs=warm[:, 128:128 + cols])

        stgs = {}
        tail_pb = [None]
        # output DMAs alternate between the gpsimd and sync queues so the
        # ~650ns per-issue serialization never piles up on one queue
        dma_n = [0]

        def out_dma(dst_ap, src_ap):
            eng = nc.gpsimd if dma_n[0] % 2 == 0 else nc.sync
            dma_n[0] += 1
            eng.dma_start(out=dst_ap, in_=src_ap)

        def emit_b_group(j, g, half=None):
            # stage-B for quarter j, 4-ROI group g: 4 col-tiled matmuls at
            # PSUM partition offsets 0/32/64/96 (concurrent execution).
            # half None: all 64 channels; 0/1: 32-channel half (224 cols),
            # used to drain most of the last quarter inside the pipeline.
            hf, jj = divmod(j, 2)
            if jj == 0:
                stgs[g] = stg_pool.tile([128, 2, 448], f16, tag="stg",
                                        name=f"stg_{hf}_{g}")
            stg = stgs[g]
            g0 = 4 * g
            rois_g = list(range(g0, min(g0 + 4, S)))
            hi = 32 * len(rois_g)
            c0, c1 = (32 * half, 32 * half + 32) if half in (0, 1) \
                else (0, 64)
            x0, x1 = c0 * POOLED, c1 * POOLED
            if half == "tail" and g % 2 == 0:
                pa_t = pa_pool.tile([128, 4, 256], f32, tag="pa")
                pb = pa_t.rearrange("h a b -> h (a b)")
            else:
                pb = pb_pool.tile([128, 512], f32, tag="pb")
            for i, s in enumerate(rois_g):
                nc.tensor.matmul(
                    pb[32 * i:32 * i + 32, 0:x1 - x0],
                    lhsT=wy_t[:, s * 32:(s + 1) * 32],
                    rhs=t1[j][:, s, c0:c1, :],
                    tile_position=(0, 32 * i),
                )
            cv, cs = (V_H, S_H) if half in (0, 1) else (V_B, S_B)
            if half == "tail":
                copy(stg[0:hi, jj, x0:x1], pb[0:hi, 0:x1 - x0], cv, cs,
                     strict=(g % 2 == 0))
            else:
                # assign to the engine of the most recent stage-A copy:
                # its next pa-critical copy is furthest away, so this
                # B-copy never delays the pa rotation from the queue head
                copy(stg[0:hi, jj, x0:x1], pb[0:hi, 0:x1 - x0], cv, cs,
                     strict=a_sched[max(a_flip[0] - 1, 0)])
            # DMA each finished piece immediately so output traffic spreads
            # across the whole pipeline instead of piling up in the tail
            out_dma(out[g, hf, 0:hi, jj, x0:x1], stg[0:hi, jj, x0:x1])

        # chunk DMAs: first four pre-issued across idle engine queues (the
        # ~790ns per-issue serialization on one queue starves stage A);
        # later chunks prefetched two ahead on sync
        fw_tiles = {}

        def issue_chunk(k, eng):
            if k >= len(chunks) or k in fw_tiles:
                return
            t = fw_pool.tile([W, CHUNK, H], f16, tag="fw")
            eng.dma_start(out=t[:, 0:chunks[k], :],
                          in_=fwt[:, starts[k]:starts[k] + chunks[k], :])
            fw_tiles[k] = t

        issue_chunk(0, nc.scalar)
        issue_chunk(1, nc.gpsimd)
        issue_chunk(2, nc.gpsimd)
        issue_chunk(3, nc.gpsimd)
        # wy is large (S*32*H fp16) and first needed by stage B a quarter
        # in; issuing it after the early chunks keeps chunk0's completion
        # (which gates the whole copy chain) off its shadow
        nc.gpsimd.dma_start(out=wy_t[:], in_=wyt[:])

        # warmup runs first (forced via high_priority) while the early
        # chunks land; the first stage-A group follows as soon as chunk0
        # is in SBUF so the copy chain (the pipeline pacer) starts early
        with tc.high_priority():
            warmup(9)
        cur = fw_tiles[0]
        coff = 0
        pa = pa_pool.tile([128, 4, 256], f32, tag="pa")
        for c in range(4):
            nc.tensor.matmul(pa[0:H, c, 0:SQ], lhsT=cur[:, c, :],
                             rhs=wx_t[:])
        copy_a(t1[0][:, :, 0:4, :],
               pa[0:H, :, 0:SQ].rearrange("h c (s q) -> h s c q",
                                          q=POOLED))

        chunk_idx = 1
        for j in range(4):
            # ---- Stage A quarter: T1_j[h, s, c', q] = F[c].T @ wx ----
            # with the previous quarter's stage-B groups interleaved every
            # few channels to fill the copy-paced gaps on the PE
            pending_b = list(range(NG)) if j > 0 else []
            for ci, c in enumerate(range(64 * j, 64 * (j + 1))):
                if ci == 0 and j == 0:
                    continue  # channels 0-3 done above
                if c == starts[chunk_idx]:
                    cur = fw_tiles[chunk_idx]
                    coff = c
                    issue_chunk(chunk_idx + 2, nc.sync)
                    chunk_idx += 1
                if c % 4 == 0:
                    pa = pa_pool.tile([128, 4, 256], f32, tag="pa")
                if j == 0 and ci < 4:
                    continue
                nc.tensor.matmul(
                    pa[0:H, c % 4, 0:SQ],
                    lhsT=cur[:, c - coff, :],
                    rhs=wx_t[:],
                )
                if c % 4 == 3:
                    src = pa[0:H, :, 0:SQ].rearrange(
                        "h c (s q) -> h s c q", q=POOLED)
                    dst = t1[j][:, :, (c - 3) % 64:(c - 3) % 64 + 4, :]
                    copy_a(dst, src)
                if ci >= 13 and (ci - 13) % 6 == 0 and pending_b:
                    emit_b_group(j - 1, pending_b.pop(0))
                elif j == 0 and ci % 8 == 4:
                    # no stage-B work yet: keepalive matmul to hold the
                    # HAM clock gate at 8/8 through the copy-paced gaps
                    keepalive(512)
            for g in pending_b:
                emit_b_group(j - 1, g)
            if j < 3:
                # bridge keepalive over the quarter boundary (the next
                # quarter's first B group waits on this quarter's last
                # copy; a PE hole here trips the HAM re-throttle)
                keepalive(256)
        # last quarter's stage B (tail)
        keepalive(256)
        for g in range(NG):
            emit_b_group(3, g, half="tail")
            if g % 3 == 2:
                keepalive()

    nc.compile()
    return nc


def _plan_shards(bi, n_rois):
    """Assign ROI indices to (wave, core, slot). Returns S and a list of
    per-wave assignment arrays of shape [N_CORES, S] (-1 = padding)."""
    groups = [np.where(bi == b)[0] for b in range(B)]
    need = max((len(g) + CORES_PER_BATCH - 1) // CORES_PER_BATCH
               for g in groups)
    need = max(need, 1)
    S = min(need, S_CAP)
    per_wave_cap = S * CORES_PER_BATCH
    n_waves = max(-(-len(g) // per_wave_cap) for g in groups)
    waves = []
    for wv in range(n_waves):
        asg = np.full((N_CORES, S), -1, dtype=np.int64)
        for b in range(B):
            g = groups[b][wv * per_wave_cap:(wv + 1) * per_wave_cap]
            for k in range(CORES_PER_BATCH):
                chunk = g[k * S:(k + 1) * S]
                asg[b * CORES_PER_BATCH + k, :len(chunk)] = chunk
        waves.append(asg)
    return S, waves


def kernel(features, rois, _trace=False):
    from concourse.bass_utils import run_bass_kernel_spmd

    features = np.asarray(features, dtype=np.float32)
    rois = np.asarray(rois, dtype=np.float32)
    n_rois = rois.shape[0]
    bi = np.rint(rois[:, 0]).astype(np.int64)
    bi = np.where((bi >= 0) & (bi < B), bi, -1)

    wy, wx = _host_weights(rois)  # [N, 7, H] / [N, 7, W], 1/area folded
    S, waves = _plan_shards(bi, n_rois)

    if S not in _prog_cache:
        _prog_cache[S] = _build_program(S)
    nc = _prog_cache[S]

    # Features per batch, transposed to [W, C, H], fp16. Shared across the
    # 4 cores of each batch group.
    fwt = [np.ascontiguousarray(features[b].transpose(2, 0, 1))
           .astype(np.float16) for b in range(B)]

    out_full = np.zeros((n_rois, C, POOLED, POOLED), dtype=np.float32)
    exec_ns = None
    for asg in waves:
        in_maps = []
        for k in range(N_CORES):
            wxt = np.zeros((W, S * POOLED), dtype=np.float16)
            wyt = np.zeros((H, S * 32), dtype=np.float16)
            for s in range(S):
                r = asg[k, s]
                if r < 0:
                    continue
                # w[n, q, w-axis] -> [w-axis, s*7+q]
                wxt[:, s * POOLED:(s + 1) * POOLED] = \
                    wx[r].T.astype(np.float16)
                wyt[:, s * 32:s * 32 + POOLED] = \
                    wy[r].T.astype(np.float16)
            in_maps.append({
                "fwt": fwt[k // CORES_PER_BATCH],
                "wxt": wxt,
                "wyt": wyt,
            })
        res = run_bass_kernel_spmd(nc, in_maps, list(range(N_CORES)),
                                   trace=_trace)
        if res.exec_time_ns is not None:
            exec_ns = max(exec_ns or 0, res.exec_time_ns)
        for k in range(N_CORES):
            arr = res.results[k]["out"]  # [NG, 2, 128, 2, 448] f16
            for s in range(S):
                r = asg[k, s]
                if r < 0:
                    continue
                g, i = divmod(s, 4)
                # [hf, p, jj, c', q] -> [hf, jj, c', p, q] -> [C, 7, 7]
                blk = (arr[g, :, 32 * i:32 * i + POOLED]
                       .reshape(2, POOLED, 2, 64, POOLED)
                       .transpose(0, 2, 3, 1, 4)
                       .reshape(C, POOLED, POOLED))
                out_full[r] = blk.astype(np.float32)

    if _trace:
        kernel.last_exec_time_ns = exec_ns
    return out_full
